# revision 13
# baseline (speedup 1.0000x reference)
"""Trainium2 Bass kernel for nn_DecoderLM_91018946936840.

4-layer pre-LN decoder (D=1024, H=16, S=1024, B=4, ff=4096) on 8 NeuronCores:
data-parallel over B (4 pair-groups) x tensor-parallel 2 (heads / ffn split),
AllReduce over each core pair after the attention out-proj and the MLP
down-proj (Megatron sharding).

Attention path (QK logits, softmax, AV) runs in float32r (full-rate fp32
streaming mode, ~1.2e-4 rounding); the wo and MLP matmuls run bf16 with fp32
PSUM accumulation. Activations are feature-major [D, tokens]: LayerNorm
statistics come from ones-matmuls, per-token scalars are broadcast across
partitions with K=1 matmuls, and each head's softmax denominator rides along
the AV matmul as a ones-column appended to V.
"""
import numpy as np
import ml_dtypes

import concourse.bass as bass
import concourse.mybir as mybir
import concourse.tile as tile
from concourse.bass_utils import run_bass_kernel_spmd
from concourse.vector_clock import ScopedClock

# ---------------------------------------------------------------------------
# Workaround: this walrus build accepts at most ONE semaphore wait per
# instruction ("Too many sync wait commands"). Redistribute Tile-assigned
# waits onto single-wait NoOps in front of the owning instruction, and do the
# same for the kernel-tail drain.
# ---------------------------------------------------------------------------
_MAX_WAITS = 1


def _patched_drain_and_barrier(self, tick_clock, wait_clock):
    nc = self.nc
    probe = nc.sync.nop(hint="drain_waits", nofuse=True)
    wait_clock.add_sem_waits(probe.ins, ScopedClock({None: tick_clock.global_clock}))
    si = probe.ins.sync_info
    waits = list(si.on_wait) if si is not None else []
    probe.ins.sync_info = mybir.SyncInfo(
        on_wait=waits[:_MAX_WAITS],
        on_update=list(si.on_update) if si is not None else [],
    )
    for i in range(_MAX_WAITS, len(waits), _MAX_WAITS):
        extra = nc.sync.nop(hint="drain_waits", nofuse=True)
        extra.ins.sync_info = mybir.SyncInfo(
            on_wait=waits[i : i + _MAX_WAITS], on_update=[])
    nc.sync.drain()
    nc.all_engine_barrier()
    assert self.sems is not None
    popped = nc._tile_sem_poison_stack.pop()
    assert popped is self._sem_poison
    nc.clear_and_free_semaphores(list(self.sems.allocated().values()))
    nc.all_engine_barrier()


_orig_commit = tile.TileContext._commit_instruction


def _patched_commit_instruction(self, inst, lazy_reg_writes=True):
    si = inst.sync_info
    if si is not None and len(si.on_wait) > _MAX_WAITS:
        waits = list(si.on_wait)
        keep, extras = waits[-_MAX_WAITS:], waits[:-_MAX_WAITS]
        engine = inst.engine
        if engine == mybir.EngineType.Unassigned:
            engine = mybir.EngineType.SP
        for w in extras:
            nop = mybir.InstNoOp(
                name=self.nc.get_next_instruction_name(),
                ins=[],
                outs=[],
                engine=engine,
                sync_info=mybir.SyncInfo(on_wait=[w], on_update=[]),
            )
            self._add_instruction(nop)
        inst.sync_info = mybir.SyncInfo(on_wait=keep, on_update=list(si.on_update))
    return _orig_commit(self, inst, lazy_reg_writes)


tile.TileContext._drain_and_barrier = _patched_drain_and_barrier
tile.TileContext._commit_instruction = _patched_commit_instruction

# ---------------------------------------------------------------------------

V, D, H, L, B, S = 32000, 1024, 16, 4, 4, 1024
HD = D // H          # 64
FF = 4 * D           # 4096
EPS = 1e-5
N_CORES = 8
SH = D // 2          # 512   qkv output shard per core (8 heads)
FSH = FF // 2        # 2048  ffn shard per core
NT = D // 128        # 8     model-dim tiles
NQ = SH // 128       # 4     shard-dim tiles
NM = FSH // 128      # 16    ffn-shard tiles
CH = 512             # token chunk
NCH = S // CH        # 2
HL = 8               # heads per core

F32 = mybir.dt.float32
F32R = mybir.dt.float32r
BF16 = mybir.dt.bfloat16
ADD = mybir.AluOpType.add
MULT = mybir.AluOpType.mult
SUB = mybir.AluOpType.subtract
AF = mybir.ActivationFunctionType

REPLICA_GROUPS = [[0, 1], [2, 3], [4, 5], [6, 7]]


def build_nc(repeat=1, skip_ar=False, skip_mlp=False, skip_attn=False, skip_qkv=False):
    nc = bass.Bass(trn_type="TRN2", target_bir_lowering=False, debug=False,
                   num_devices=N_CORES)

    def inp(name, shape, dt=F32):
        return nc.dram_tensor(name, list(shape), dt, kind="ExternalInput")

    x0t = inp("x0t", [D, S])
    wq_d = inp("wq", [L, D, SH])
    wk_d = inp("wk", [L, D, SH])
    wv_d = inp("wv", [L, D, SH])
    wo_d = inp("wo", [L, SH, D], BF16)
    w1_d = inp("w1", [L, D, FSH], BF16)
    w2_d = inp("w2", [L, FSH, D], BF16)
    bq_d = inp("bq", [L, 128, NQ])
    bk_d = inp("bk", [L, 128, NQ])
    bv_d = inp("bv", [L, 128, NQ])
    bo_d = inp("bo2", [L, 128, NT])     # pre-halved
    b1_d = inp("b1", [L, 128, NM])
    b2_d = inp("b22", [L, 128, NT])     # pre-halved
    g1_d = inp("g1", [L, 128, NT])
    be1_d = inp("be1", [L, 128, NT])
    g2_d = inp("g2", [L, 128, NT])
    be2_d = inp("be2", [L, 128, NT])
    gf_d = inp("gf", [128, NT])
    bef_d = inp("bef", [128, NT])
    mask_d = inp("masks", [128, 4, CH])
    ones_d = inp("cones", [128, 128])

    out_ext = nc.dram_tensor("outt", [D, S], F32, kind="ExternalOutput")
    out_v = out_ext.ap().rearrange("(t p) s -> p t s", p=128)

    with tile.TileContext(nc) as tc:
        with (
            nc.allow_low_precision(reason="bf16 wo/mlp + f32r matmuls"),
            tc.tile_pool(name="singles", bufs=1) as singles,
            tc.tile_pool(name="acts", bufs=1) as acts,
            tc.tile_pool(name="hg", bufs=1) as hg,
            tc.tile_pool(name="ws", bufs=6) as wstream,
            tc.tile_pool(name="wt", bufs=8) as wtp,
            tc.tile_pool(name="rows", bufs=2) as rows,
            tc.tile_pool(name="evac", bufs=3) as evac,
            tc.tile_pool(name="pp", bufs=4, space="PSUM") as pp,
            tc.tile_pool(name="pav", bufs=2, space="PSUM") as pav,
            tc.tile_pool(name="prow", bufs=1, space="PSUM") as prow,
            tc.tile_pool(name="dram", bufs=4, space="DRAM") as dram,
        ):
            # ---- resident constants -------------------------------------
            xT = singles.tile([128, NT, S], F32R)
            nc.sync.dma_start(
                out=xT[:],
                in_=x0t.ap().rearrange("(t p) s -> p t s", p=128).bitcast(F32R))
            masks = singles.tile([128, 4, CH], BF16)
            nc.gpsimd.dma_start(out=masks[:], in_=mask_d.ap())
            onesr = singles.tile([128, 128], F32R)
            nc.sync.dma_start(out=onesr[:], in_=ones_d.ap().bitcast(F32R))
            eps_t = singles.tile([1, 1], F32)
            nc.vector.memset(eps_t[:], EPS)

            def load_pp(d, shape):
                t = singles.tile(list(shape), F32, name=f"pp_{d.name}")
                src = d.ap()
                if len(shape) == 3:
                    src = src.rearrange("l p m -> p l m")
                nc.sync.dma_start(out=t[:], in_=src)
                return t

            bqT = load_pp(bq_d, [128, L, NQ])
            bkT = load_pp(bk_d, [128, L, NQ])
            bvT = load_pp(bv_d, [128, L, NQ])
            boT = load_pp(bo_d, [128, L, NT])
            b1T = load_pp(b1_d, [128, L, NM])
            b2T = load_pp(b2_d, [128, L, NT])
            g1T = load_pp(g1_d, [128, L, NT])
            be1T = load_pp(be1_d, [128, L, NT])
            g2T = load_pp(g2_d, [128, L, NT])
            be2T = load_pp(be2_d, [128, L, NT])
            gfT = load_pp(gf_d, [128, NT])
            befT = load_pp(bef_d, [128, NT])

            ones_k = onesr[:, 0:1]            # [128,1] lhsT for column sums
            ones_bc = onesr[0:1, :]           # [1,128] lhsT for broadcasts

            # ---- layernorm helper ---------------------------------------
            def layernorm(gT, bT, l_idx, dest_fn):
                """LN over the feature dim of xT (feature-major).

                dest_fn(t, cs, tmp2_ap, g_slice, b_slice) writes the result.
                """
                for c in range(NCH):
                    cs = slice(c * CH, (c + 1) * CH)
                    psx = prow.tile([1, CH], F32, tag="psx")
                    psx2 = prow.tile([1, CH], F32, tag="psx2")
                    for t in range(NT):
                        nc.tensor.matmul(psx[:], ones_k, xT[:, t, cs],
                                         start=(t == 0), stop=(t == NT - 1))
                    for t in range(NT):
                        sq = evac.tile([128, CH], F32R, tag="lntmp")
                        nc.vector.tensor_tensor(out=sq[:], in0=xT[:, t, cs],
                                                in1=xT[:, t, cs], op=MULT)
                        nc.tensor.matmul(psx2[:], ones_k, sq[:],
                                         start=(t == 0), stop=(t == NT - 1))
                    mrow = rows.tile([1, CH], F32R, tag="mr")
                    nc.scalar.mul(out=mrow[:], in_=psx[:], mul=1.0 / D)
                    m2row = rows.tile([1, CH], F32R, tag="rb")
                    nc.vector.tensor_tensor(out=m2row[:], in0=mrow[:],
                                            in1=mrow[:], op=MULT)
                    vrow = rows.tile([1, CH], F32R, tag="rb")
                    nc.scalar.mul(out=vrow[:], in_=psx2[:], mul=1.0 / D)
                    nc.vector.tensor_tensor(out=vrow[:], in0=vrow[:],
                                            in1=m2row[:], op=SUB)
                    srow = rows.tile([1, CH], F32R, tag="rb")
                    nc.scalar.activation(out=srow[:], in_=vrow[:],
                                         func=AF.Sqrt, bias=eps_t[:], scale=1.0)
                    rrow = rows.tile([1, CH], F32R, tag="rrow")
                    nc.vector.reciprocal(out=rrow[:], in_=srow[:])
                    mrrow = rows.tile([1, CH], F32R, tag="mr")
                    nc.vector.tensor_tensor(out=mrrow[:], in0=mrow[:],
                                            in1=rrow[:], op=MULT)
                    prb = pp.tile([128, CH], F32, tag="pp")
                    nc.tensor.matmul(prb[:], ones_bc, rrow[:],
                                     start=True, stop=True)
                    pmrb = pp.tile([128, CH], F32, tag="pp")
                    nc.tensor.matmul(pmrb[:], ones_bc, mrrow[:],
                                     start=True, stop=True)
                    for t in range(NT):
                        tmp = evac.tile([128, CH], F32R, tag="lntmp")
                        nc.vector.tensor_tensor(out=tmp[:], in0=xT[:, t, cs],
                                                in1=prb[:], op=MULT)
                        nc.vector.tensor_tensor(out=tmp[:], in0=tmp[:],
                                                in1=pmrb[:], op=SUB)
                        if l_idx is not None:
                            gs = gT[:, l_idx, t : t + 1]
                            bs = bT[:, l_idx, t : t + 1]
                        else:
                            gs = gT[:, t : t + 1]
                            bs = bT[:, t : t + 1]
                        dest_fn(t, cs, tmp, gs, bs)

            def wtile(dram_t, l, m, kt, dt):
                """stream one weight m-tile [128, kt, 128]"""
                w = wstream.tile([128, kt, 128], dt, tag="ws", bufs=4)
                src = dram_t.ap()[l, :, m * 128:(m + 1) * 128] \
                    .rearrange("(t p) m -> p t m", p=128)
                if dt == F32R:
                    src = src.bitcast(F32R)
                nc.sync.dma_start(out=w[:], in_=src)
                return w

            # ---- main body ----------------------------------------------
            for rep in range(repeat):
                for l in range(L):
                    # LN1 -> hT (f32r)
                    hT = hg.tile([128, NT, S], F32R, tag="hg")

                    def wr_h(t, cs, tmp, gs, bs, hT=hT):
                        nc.vector.tensor_scalar(
                            out=hT[:, t, cs], in0=tmp[:],
                            scalar1=gs, scalar2=bs, op0=MULT, op1=ADD)

                    layernorm(g1T, be1T, l, wr_h)

                    # K projection -> feature-major [128, NQ, S] f32r
                    KT = acts.tile([128, NQ, S], F32R, tag="kt")
                    for m in ([] if skip_qkv else range(NQ)):
                        wl = wtile(wk_d, l, m, NT, F32R)
                        for c in range(NCH):
                            cs = slice(c * CH, (c + 1) * CH)
                            ps = pp.tile([128, CH], F32, tag="pp")
                            for k in range(NT):
                                nc.tensor.matmul(
                                    ps[:], wl[:, k, :], hT[:, k, cs],
                                    start=(k == 0), stop=(k == NT - 1))
                            nc.vector.tensor_scalar_add(
                                out=KT[:, m, cs], in0=ps[:],
                                scalar1=bkT[:, l, m : m + 1])

                    # V projection -> token-major stripes [128, NT, 8*65] f32r
                    Vt = acts.tile([128, NT, HL * 65], BF16, tag="vt")
                    nc.gpsimd.dma_start(
                        out=Vt[:].rearrange("p t (h c) -> p t h c", h=HL)[:, :, :, 64:65],
                        in_=ones_d.ap()[:, 0 : NT * HL]
                            .rearrange("p (t h o) -> p t h o", t=NT, h=HL))
                    wvl = wstream.tile([128, NT, SH], F32R, tag="wv", bufs=1)
                    nc.sync.dma_start(
                        out=wvl[:],
                        in_=wv_d.ap()[l].rearrange("(t p) m -> p t m", p=128)
                            .bitcast(F32R))
                    for mt in ([] if skip_qkv else range(NT)):
                        ps = pp.tile([128, SH], F32, tag="pp")
                        for k in range(NT):
                            nc.tensor.matmul(
                                ps[:], hT[:, k, mt * 128:(mt + 1) * 128],
                                wvl[:, k, :],
                                start=(k == 0), stop=(k == NT - 1))
                        nc.vector.tensor_copy(
                            out=Vt[:, mt, :].rearrange("p (h c) -> p h c", h=HL)[:, :, 0:64],
                            in_=ps[:].rearrange("p (h c) -> p h c", h=HL))

                    # attention -> attnT [128, NQ, S] bf16
                    attnT = acts.tile([128, NQ, S], BF16, tag="at")
                    for c in ([] if skip_attn else range(NCH)):
                        cs = slice(c * CH, (c + 1) * CH)
                        nk = 4 * c + 4
                        QTc = acts.tile([128, NQ, CH], F32R, tag="qt", bufs=2)
                        for m in range(NQ):
                            wl = wtile(wq_d, l, m, NT, F32R)
                            ps = pp.tile([128, CH], F32, tag="pp")
                            for k in range(NT):
                                nc.tensor.matmul(
                                    ps[:], wl[:, k, :], hT[:, k, cs],
                                    start=(k == 0), stop=(k == NT - 1))
                            nc.vector.tensor_scalar_add(
                                out=QTc[:, m, :], in0=ps[:],
                                scalar1=bqT[:, l, m : m + 1])
                        for h in range(HL):
                            base = 64 * (h % 2)
                            hp = h // 2
                            wts = []
                            for j in range(nk):
                                pl = pp.tile([128, CH], F32, tag="pp")
                                nc.tensor.matmul(
                                    pl[:],
                                    KT[base:base + 64, hp, j * 128:(j + 1) * 128],
                                    QTc[base:base + 64, hp, :],
                                    start=True, stop=True)
                                wt = wtp.tile([128, CH], BF16, tag="wt", bufs=8)
                                nc.scalar.activation(out=wt[:], in_=pl[:],
                                                     func=AF.Exp, scale=0.125)
                                r = j - 4 * c
                                if r >= 0:
                                    nc.vector.tensor_tensor(
                                        out=wt[:], in0=wt[:],
                                        in1=masks[:, r, :], op=MULT)
                                wts.append(wt)
                            pa = pav.tile([65, CH], F32, tag="pav")
                            for j in range(nk):
                                nc.tensor.matmul(
                                    pa[:], Vt[:, j, 65 * h : 65 * h + 65],
                                    wts[j][:],
                                    start=(j == 0), stop=(j == nk - 1))
                            rec = rows.tile([1, CH], F32R, tag="rb")
                            nc.vector.reciprocal(out=rec[:], in_=pa[64:65, :])
                            prb2 = pp.tile([128, CH], F32, tag="pp")
                            nc.tensor.matmul(prb2[0:64, :], ones_bc[:, 0:64],
                                             rec[:], start=True, stop=True)
                            avs = wtp.tile([64, CH], BF16, tag="avs", bufs=3)
                            nc.scalar.copy(out=avs[:], in_=pa[0:64, :])
                            nc.vector.tensor_tensor(
                                out=attnT[base:base + 64, hp, cs],
                                in0=avs[:], in1=prb2[0:64, :], op=MULT)
                    for t in range(NQ):
                        nc.vector.tensor_scalar_add(
                            out=attnT[:, t, :], in0=attnT[:, t, :],
                            scalar1=bvT[:, l, t : t + 1])

                    # wo + AllReduce + residual
                    for c in range(NCH):
                        cs = slice(c * CH, (c + 1) * CH)
                        ar_in = dram.tile([128, NT, CH], BF16, tag="arin")
                        ar_out = dram.tile([128, NT, CH], BF16, tag="arout")
                        for m in range(NT):
                            wl = wtile(wo_d, l, m, NQ, BF16)
                            ps = pp.tile([128, CH], F32, tag="pp")
                            for k in range(NQ):
                                nc.tensor.matmul(
                                    ps[:], wl[:, k, :], attnT[:, k, cs],
                                    start=(k == 0), stop=(k == NQ - 1))
                            po = evac.tile([128, CH], BF16, tag="po", bufs=2)
                            nc.vector.tensor_scalar_add(
                                out=po[:], in0=ps[:],
                                scalar1=boT[:, l, m : m + 1])
                            nc.sync.dma_start(out=ar_in[:, m, :], in_=po[:])
                        if not skip_ar:
                            nc.gpsimd.collective_compute(
                                "AllReduce", ADD, replica_groups=REPLICA_GROUPS,
                                ins=[ar_in.opt()], outs=[ar_out.opt()])
                        else:
                            nc.sync.dma_start(out=ar_out[:], in_=ar_in[:])
                        for t in range(NT):
                            ars = evac.tile([128, CH], BF16, tag="ars", bufs=2)
                            nc.sync.dma_start(out=ars[:],
                                              in_=ar_out[:, t, :])
                            nc.vector.tensor_tensor(
                                out=xT[:, t, cs], in0=xT[:, t, cs],
                                in1=ars[:], op=ADD)

                    # LN2 -> h2 chunks (bf16)
                    h2c = {}

                    def wr_h2(t, cs, tmp, gs, bs, h2c=h2c):
                        c = cs.start // CH
                        if c not in h2c:
                            h2c[c] = acts.tile([128, NT, CH], BF16,
                                               tag="h2", bufs=2, name=f"h2c{c}")
                        nc.vector.tensor_scalar(
                            out=h2c[c][:, t, :], in0=tmp[:],
                            scalar1=gs, scalar2=bs, op0=MULT, op1=ADD)

                    layernorm(g2T, be2T, l, wr_h2)

                    # MLP
                    gT = hg.tile([128, NM, S], BF16, tag="hg")
                    for c in ([] if skip_mlp else range(NCH)):
                        cs = slice(c * CH, (c + 1) * CH)
                        for m in range(NM):
                            wl = wtile(w1_d, l, m, NT, BF16)
                            ps = pp.tile([128, CH], F32, tag="pp")
                            for k in range(NT):
                                nc.tensor.matmul(
                                    ps[:], wl[:, k, :], h2c[c][:, k, :],
                                    start=(k == 0), stop=(k == NT - 1))
                            nc.scalar.activation(
                                out=gT[:, m, cs], in_=ps[:], func=AF.Gelu,
                                bias=b1T[:, l, m : m + 1], scale=1.0)
                        ar_in = dram.tile([128, NT, CH], BF16, tag="arin")
                        ar_out = dram.tile([128, NT, CH], BF16, tag="arout")
                        for m in range(NT):
                            wl = wtile(w2_d, l, m, NM, BF16)
                            ps = pp.tile([128, CH], F32, tag="pp")
                            for k in range(NM):
                                nc.tensor.matmul(
                                    ps[:], wl[:, k, :], gT[:, k, cs],
                                    start=(k == 0), stop=(k == NM - 1))
                            po = evac.tile([128, CH], BF16, tag="po", bufs=2)
                            nc.vector.tensor_scalar_add(
                                out=po[:], in0=ps[:],
                                scalar1=b2T[:, l, m : m + 1])
                            nc.sync.dma_start(out=ar_in[:, m, :], in_=po[:])
                        if not skip_ar:
                            nc.gpsimd.collective_compute(
                                "AllReduce", ADD, replica_groups=REPLICA_GROUPS,
                                ins=[ar_in.opt()], outs=[ar_out.opt()])
                        else:
                            nc.sync.dma_start(out=ar_out[:], in_=ar_in[:])
                        for t in range(NT):
                            ars = evac.tile([128, CH], BF16, tag="ars", bufs=2)
                            nc.sync.dma_start(out=ars[:],
                                              in_=ar_out[:, t, :])
                            nc.vector.tensor_tensor(
                                out=xT[:, t, cs], in0=xT[:, t, cs],
                                in1=ars[:], op=ADD)

            # final LN -> output (streamed per tile)
            def wr_out(t, cs, tmp, gs, bs):
                ot = evac.tile([128, CH], F32, tag="ot", bufs=2)
                nc.vector.tensor_scalar(out=ot[:], in0=tmp[:],
                                        scalar1=gs, scalar2=bs,
                                        op0=MULT, op1=ADD)
                nc.sync.dma_start(out=out_v[:, t, cs], in_=ot[:])

            layernorm(gfT, befT, None, wr_out)

    return nc


# ---------------------------------------------------------------------------
# host side
# ---------------------------------------------------------------------------

def _sinusoidal_pe(s, d):
    pos = np.arange(s, dtype=np.float32)[:, None]
    div = np.exp(np.arange(0, d, 2, dtype=np.float32)
                 * np.float32(-np.log(10000.0) / d)).astype(np.float32)
    pe = np.zeros((s, d), dtype=np.float32)
    pe[:, 0::2] = np.sin(pos * div)
    pe[:, 1::2] = np.cos(pos * div)
    return pe


def _pp128(v):
    """[L?, n*128] -> [L?, 128, n] with feature = 128*m + p."""
    v = np.asarray(v, dtype=np.float32)
    if v.ndim == 1:
        return np.ascontiguousarray(v.reshape(-1, 128).T)
    lq, n = v.shape
    return np.ascontiguousarray(v.reshape(lq, n // 128, 128).transpose(0, 2, 1))


_NC_CACHE = {}


def _get_nc(repeat=1):
    if repeat not in _NC_CACHE:
        _NC_CACHE[repeat] = build_nc(repeat)
    return _NC_CACHE[repeat]


def make_in_maps(input_ids, tok_emb, wq, bq, wk, bk, wv, bv, wo, bo,
                 ln1_g, ln1_b, ln2_g, ln2_b, w1, b1, w2, b2, lnf_g, lnf_b):
    input_ids = np.asarray(input_ids)
    pe = _sinusoidal_pe(S, D)
    masks = np.zeros((128, 4, CH), dtype=np.float32)
    ar = np.arange(CH)
    for r in range(4):
        for p in range(128):
            masks[p, r, :] = (ar >= 128 * r + p).astype(np.float32)
    cones = np.ones((128, 128), dtype=np.float32)

    in_maps = []
    for core in range(N_CORES):
        b = core // 2
        j = core % 2
        qs = slice(j * SH, (j + 1) * SH)
        fs = slice(j * FSH, (j + 1) * FSH)
        x0 = (tok_emb[input_ids[b]] + pe).astype(np.float32)   # [S, D]
        m = {
            "x0t": np.ascontiguousarray(x0.T),
            "wq": np.ascontiguousarray(wq[:, :, qs]),
            "wk": np.ascontiguousarray(wk[:, :, qs]),
            "wv": np.ascontiguousarray(wv[:, :, qs]),
            "wo": np.ascontiguousarray(wo[:, qs, :]).astype(ml_dtypes.bfloat16),
            "w1": np.ascontiguousarray(w1[:, :, fs]).astype(ml_dtypes.bfloat16),
            "w2": np.ascontiguousarray(w2[:, fs, :]).astype(ml_dtypes.bfloat16),
            "bq": _pp128(bq[:, qs]),
            "bk": _pp128(bk[:, qs]),
            "bv": _pp128(bv[:, qs]),
            "bo2": _pp128(bo * 0.5),
            "b1": _pp128(b1[:, fs]),
            "b22": _pp128(b2 * 0.5),
            "g1": _pp128(ln1_g),
            "be1": _pp128(ln1_b),
            "g2": _pp128(ln2_g),
            "be2": _pp128(ln2_b),
            "gf": _pp128(lnf_g),
            "bef": _pp128(lnf_b),
            "masks": masks,
            "cones": cones,
        }
        in_maps.append(m)
    return in_maps


def kernel(input_ids, attention_mask, tok_emb, ln1_g, ln1_b, wq, bq, wk, bk,
           wv, bv, wo, bo, ln2_g, ln2_b, w1, b1, w2, b2, lnf_g, lnf_b,
           _repeat=1):
    args = [np.asarray(a, dtype=np.float32) for a in
            (tok_emb, wq, bq, wk, bk, wv, bv, wo, bo,
             ln1_g, ln1_b, ln2_g, ln2_b, w1, b1, w2, b2, lnf_g, lnf_b)]
    (tok_emb, wq, bq, wk, bk, wv, bv, wo, bo,
     ln1_g, ln1_b, ln2_g, ln2_b, w1, b1, w2, b2, lnf_g, lnf_b) = args
    in_maps = make_in_maps(input_ids, tok_emb, wq, bq, wk, bk, wv, bv, wo, bo,
                           ln1_g, ln1_b, ln2_g, ln2_b, w1, b1, w2, b2,
                           lnf_g, lnf_b)
    nc = _get_nc(_repeat)
    res = run_bass_kernel_spmd(nc, in_maps, list(range(N_CORES)))
    out = np.empty((B, S, D), dtype=np.float32)
    for b in range(B):
        out[b] = res.results[2 * b]["outt"].T
    return out


# revision 18
# speedup vs baseline: 336.5819x; 336.5819x over previous
"""Trainium2 Bass kernel for nn_DecoderLM_91018946936840.

4-layer pre-LN decoder (D=1024, H=16, S=1024, B=4, ff=4096) on 8 NeuronCores:
data-parallel over B (4 pair-groups) x tensor-parallel 2 (heads / ffn split),
AllReduce over each core pair after the attention out-proj and the MLP
down-proj (Megatron sharding).

Attention path (QK logits, softmax, AV) runs in float32r (full-rate fp32
streaming mode, ~1.2e-4 rounding); the wo and MLP matmuls run bf16 with fp32
PSUM accumulation. Activations are feature-major [D, tokens]: LayerNorm
statistics come from ones-matmuls, per-token scalars are broadcast across
partitions with K=1 matmuls, and each head's softmax denominator rides along
the AV matmul as a ones-column appended to V.
"""
import numpy as np
import ml_dtypes

import concourse.bass as bass
import concourse.mybir as mybir
import concourse.tile as tile
from concourse.bass_utils import run_bass_kernel_spmd
from concourse.vector_clock import ScopedClock

# ---------------------------------------------------------------------------
# Workaround: this walrus build accepts at most ONE semaphore wait per
# instruction ("Too many sync wait commands"). Redistribute Tile-assigned
# waits onto single-wait NoOps in front of the owning instruction, and do the
# same for the kernel-tail drain.
# ---------------------------------------------------------------------------
_MAX_WAITS = 1


def _patched_drain_and_barrier(self, tick_clock, wait_clock):
    nc = self.nc
    probe = nc.sync.nop(hint="drain_waits", nofuse=True)
    wait_clock.add_sem_waits(probe.ins, ScopedClock({None: tick_clock.global_clock}))
    si = probe.ins.sync_info
    waits = list(si.on_wait) if si is not None else []
    probe.ins.sync_info = mybir.SyncInfo(
        on_wait=waits[:_MAX_WAITS],
        on_update=list(si.on_update) if si is not None else [],
    )
    for i in range(_MAX_WAITS, len(waits), _MAX_WAITS):
        extra = nc.sync.nop(hint="drain_waits", nofuse=True)
        extra.ins.sync_info = mybir.SyncInfo(
            on_wait=waits[i : i + _MAX_WAITS], on_update=[])
    nc.sync.drain()
    nc.all_engine_barrier()
    assert self.sems is not None
    popped = nc._tile_sem_poison_stack.pop()
    assert popped is self._sem_poison
    nc.clear_and_free_semaphores(list(self.sems.allocated().values()))
    nc.all_engine_barrier()


_orig_commit = tile.TileContext._commit_instruction


def _patched_commit_instruction(self, inst, lazy_reg_writes=True):
    si = inst.sync_info
    if si is not None and len(si.on_wait) > _MAX_WAITS:
        waits = list(si.on_wait)
        keep, extras = waits[-_MAX_WAITS:], waits[:-_MAX_WAITS]
        engine = inst.engine
        if engine == mybir.EngineType.Unassigned:
            engine = mybir.EngineType.SP
        for w in extras:
            nop = mybir.InstNoOp(
                name=self.nc.get_next_instruction_name(),
                ins=[],
                outs=[],
                engine=engine,
                sync_info=mybir.SyncInfo(on_wait=[w], on_update=[]),
            )
            self._add_instruction(nop)
        inst.sync_info = mybir.SyncInfo(on_wait=keep, on_update=list(si.on_update))
    return _orig_commit(self, inst, lazy_reg_writes)


tile.TileContext._drain_and_barrier = _patched_drain_and_barrier
tile.TileContext._commit_instruction = _patched_commit_instruction

# ---------------------------------------------------------------------------

V, D, H, L, B, S = 32000, 1024, 16, 4, 4, 1024
HD = D // H          # 64
FF = 4 * D           # 4096
EPS = 1e-5
N_CORES = 8
SH = D // 2          # 512   qkv output shard per core (8 heads)
FSH = FF // 2        # 2048  ffn shard per core
NT = D // 128        # 8     model-dim tiles
NQ = SH // 128       # 4     shard-dim tiles
NM = FSH // 128      # 16    ffn-shard tiles
CH = 512             # token chunk
NCH = S // CH        # 2
HL = 8               # heads per core

F32 = mybir.dt.float32
F32R = mybir.dt.float32r
BF16 = mybir.dt.bfloat16
ADD = mybir.AluOpType.add
MULT = mybir.AluOpType.mult
SUB = mybir.AluOpType.subtract
AF = mybir.ActivationFunctionType

REPLICA_GROUPS = [[0, 1], [2, 3], [4, 5], [6, 7]]


def build_nc(repeat=1, skip_ar=False, skip_mlp=False, skip_attn=False, skip_qkv=False):
    nc = bass.Bass(trn_type="TRN2", target_bir_lowering=False, debug=False,
                   num_devices=N_CORES)

    def inp(name, shape, dt=F32):
        return nc.dram_tensor(name, list(shape), dt, kind="ExternalInput")

    x0t = inp("x0t", [D, S])
    wq_d = inp("wq", [L, D, SH])
    wk_d = inp("wk", [L, D, SH])
    wv_d = inp("wv", [L, D, SH])
    wo_d = inp("wo", [L, SH, D], BF16)
    w1_d = inp("w1", [L, D, FSH], BF16)
    w2_d = inp("w2", [L, FSH, D], BF16)
    bq_d = inp("bq", [L, 128, NQ])
    bk_d = inp("bk", [L, 128, NQ])
    bv_d = inp("bv", [L, 128, NQ])
    bo_d = inp("bo2", [L, 128, NT])     # pre-halved
    b1_d = inp("b1", [L, 128, NM])
    b2_d = inp("b22", [L, 128, NT])     # pre-halved
    g1_d = inp("g1", [L, 128, NT])
    be1_d = inp("be1", [L, 128, NT])
    g2_d = inp("g2", [L, 128, NT])
    be2_d = inp("be2", [L, 128, NT])
    gf_d = inp("gf", [128, NT])
    bef_d = inp("bef", [128, NT])
    mask_d = inp("masks", [128, 4, CH])
    ones_d = inp("cones", [128, 128])

    out_ext = nc.dram_tensor("outt", [D, S], F32, kind="ExternalOutput")
    out_v = out_ext.ap().rearrange("(t p) s -> p t s", p=128)

    with tile.TileContext(nc) as tc:
        with (
            nc.allow_low_precision(reason="bf16 wo/mlp + f32r matmuls"),
            tc.tile_pool(name="singles", bufs=1) as singles,
            tc.tile_pool(name="acts", bufs=1) as acts,
            tc.tile_pool(name="hg", bufs=1) as hg,
            tc.tile_pool(name="ws", bufs=6) as wstream,
            tc.tile_pool(name="wt", bufs=8) as wtp,
            tc.tile_pool(name="rows", bufs=2) as rows,
            tc.tile_pool(name="evac", bufs=3) as evac,
            tc.tile_pool(name="pp", bufs=4, space="PSUM") as pp,
            tc.tile_pool(name="pav", bufs=2, space="PSUM") as pav,
            tc.tile_pool(name="prow", bufs=1, space="PSUM") as prow,
            tc.tile_pool(name="dram", bufs=4, space="DRAM") as dram,
        ):
            # ---- resident constants -------------------------------------
            xT = singles.tile([128, NT, S], F32R)
            nc.sync.dma_start(
                out=xT[:],
                in_=x0t.ap().rearrange("(t p) s -> p t s", p=128).bitcast(F32R))
            masks = singles.tile([128, 4, CH], BF16)
            nc.gpsimd.dma_start(out=masks[:], in_=mask_d.ap())
            onesr = singles.tile([128, 128], F32R)
            nc.sync.dma_start(out=onesr[:], in_=ones_d.ap().bitcast(F32R))
            eps_t = singles.tile([1, 1], F32)
            nc.vector.memset(eps_t[:], EPS)

            def load_pp(d, shape):
                t = singles.tile(list(shape), F32, name=f"pp_{d.name}")
                src = d.ap()
                if len(shape) == 3:
                    src = src.rearrange("l p m -> p l m")
                nc.sync.dma_start(out=t[:], in_=src)
                return t

            bqT = load_pp(bq_d, [128, L, NQ])
            bkT = load_pp(bk_d, [128, L, NQ])
            bvT = load_pp(bv_d, [128, L, NQ])
            boT = load_pp(bo_d, [128, L, NT])
            b1T = load_pp(b1_d, [128, L, NM])
            b2T = load_pp(b2_d, [128, L, NT])
            g1T = load_pp(g1_d, [128, L, NT])
            be1T = load_pp(be1_d, [128, L, NT])
            g2T = load_pp(g2_d, [128, L, NT])
            be2T = load_pp(be2_d, [128, L, NT])
            gfT = load_pp(gf_d, [128, NT])
            befT = load_pp(bef_d, [128, NT])

            ones_k = onesr[:, 0:1]            # [128,1] lhsT for column sums
            ones_bc = onesr[0:1, :]           # [1,128] lhsT for broadcasts

            # ---- layernorm helper ---------------------------------------
            def layernorm_chunk(gT, bT, l_idx, dest_fn, c):
                """LN over the feature dim of xT (feature-major), one chunk.

                dest_fn(t, cs, tmp2_ap, g_slice, b_slice) writes the result.
                """
                if True:
                    cs = slice(c * CH, (c + 1) * CH)
                    psx = prow.tile([1, CH], F32, tag="psx")
                    psx2 = prow.tile([1, CH], F32, tag="psx2")
                    for t in range(NT):
                        nc.tensor.matmul(psx[:], ones_k, xT[:, t, cs],
                                         start=(t == 0), stop=(t == NT - 1))
                    for t in range(NT):
                        sq = evac.tile([128, CH], F32R, tag="lntmp")
                        nc.vector.tensor_tensor(out=sq[:], in0=xT[:, t, cs],
                                                in1=xT[:, t, cs], op=MULT)
                        nc.tensor.matmul(psx2[:], ones_k, sq[:],
                                         start=(t == 0), stop=(t == NT - 1))
                    mrow = rows.tile([1, CH], F32R, tag="mr")
                    nc.scalar.mul(out=mrow[:], in_=psx[:], mul=1.0 / D)
                    m2row = rows.tile([1, CH], F32R, tag="rb")
                    nc.vector.tensor_tensor(out=m2row[:], in0=mrow[:],
                                            in1=mrow[:], op=MULT)
                    vrow = rows.tile([1, CH], F32R, tag="rb")
                    nc.scalar.mul(out=vrow[:], in_=psx2[:], mul=1.0 / D)
                    nc.vector.tensor_tensor(out=vrow[:], in0=vrow[:],
                                            in1=m2row[:], op=SUB)
                    srow = rows.tile([1, CH], F32R, tag="rb")
                    nc.scalar.activation(out=srow[:], in_=vrow[:],
                                         func=AF.Sqrt, bias=eps_t[:], scale=1.0)
                    rrow = rows.tile([1, CH], F32R, tag="rrow")
                    nc.vector.reciprocal(out=rrow[:], in_=srow[:])
                    mrrow = rows.tile([1, CH], F32R, tag="mr")
                    nc.vector.tensor_tensor(out=mrrow[:], in0=mrow[:],
                                            in1=rrow[:], op=MULT)
                    prb = pp.tile([128, CH], F32, tag="pp")
                    nc.tensor.matmul(prb[:], ones_bc, rrow[:],
                                     start=True, stop=True)
                    pmrb = pp.tile([128, CH], F32, tag="pp")
                    nc.tensor.matmul(pmrb[:], ones_bc, mrrow[:],
                                     start=True, stop=True)
                    for t in range(NT):
                        tmp = evac.tile([128, CH], F32R, tag="lntmp")
                        nc.vector.tensor_tensor(out=tmp[:], in0=xT[:, t, cs],
                                                in1=prb[:], op=MULT)
                        nc.vector.tensor_tensor(out=tmp[:], in0=tmp[:],
                                                in1=pmrb[:], op=SUB)
                        if l_idx is not None:
                            gs = gT[:, l_idx, t : t + 1]
                            bs = bT[:, l_idx, t : t + 1]
                        else:
                            gs = gT[:, t : t + 1]
                            bs = bT[:, t : t + 1]
                        dest_fn(t, cs, tmp, gs, bs)

            def layernorm(gT, bT, l_idx, dest_fn):
                for c in range(NCH):
                    layernorm_chunk(gT, bT, l_idx, dest_fn, c)

            def wtile(dram_t, l, m, kt, dt):
                """stream one weight m-tile [128, kt, 128]"""
                w = wstream.tile([128, kt, 128], dt, tag="ws", bufs=4)
                src = dram_t.ap()[l, :, m * 128:(m + 1) * 128] \
                    .rearrange("(t p) m -> p t m", p=128)
                if dt == F32R:
                    src = src.bitcast(F32R)
                nc.sync.dma_start(out=w[:], in_=src)
                return w

            # ---- main body ----------------------------------------------
            for rep in range(repeat):
                for l in range(L):
                    # LN1 -> hT (f32r)
                    hT = hg.tile([128, NT, S], F32R, tag="hg")

                    def wr_h(t, cs, tmp, gs, bs, hT=hT):
                        nc.vector.tensor_scalar(
                            out=hT[:, t, cs], in0=tmp[:],
                            scalar1=gs, scalar2=bs, op0=MULT, op1=ADD)

                    layernorm(g1T, be1T, l, wr_h)

                    # K projection -> feature-major [128, NQ, S] f32r
                    KT = acts.tile([128, NQ, S], F32R, tag="kt")
                    for m in ([] if skip_qkv else range(NQ)):
                        wl = wtile(wk_d, l, m, NT, F32R)
                        for c in range(NCH):
                            cs = slice(c * CH, (c + 1) * CH)
                            ps = pp.tile([128, CH], F32, tag="pp")
                            for k in range(NT):
                                nc.tensor.matmul(
                                    ps[:], wl[:, k, :], hT[:, k, cs],
                                    start=(k == 0), stop=(k == NT - 1))
                            nc.vector.tensor_scalar_add(
                                out=KT[:, m, cs], in0=ps[:],
                                scalar1=bkT[:, l, m : m + 1])

                    # V projection -> token-major stripes [128, NT, 8*65] f32r
                    Vt = acts.tile([128, NT, HL * 65], BF16, tag="vt")
                    nc.gpsimd.dma_start(
                        out=Vt[:].rearrange("p t (h c) -> p t h c", h=HL)[:, :, :, 64:65],
                        in_=ones_d.ap()[:, 0 : NT * HL]
                            .rearrange("p (t h o) -> p t h o", t=NT, h=HL))
                    wvl = wstream.tile([128, NT, SH], F32R, tag="wv", bufs=1)
                    nc.sync.dma_start(
                        out=wvl[:],
                        in_=wv_d.ap()[l].rearrange("(t p) m -> p t m", p=128)
                            .bitcast(F32R))
                    for mt in ([] if skip_qkv else range(NT)):
                        ps = pp.tile([128, SH], F32, tag="pp")
                        for k in range(NT):
                            nc.tensor.matmul(
                                ps[:], hT[:, k, mt * 128:(mt + 1) * 128],
                                wvl[:, k, :],
                                start=(k == 0), stop=(k == NT - 1))
                        nc.vector.tensor_copy(
                            out=Vt[:, mt, :].rearrange("p (h c) -> p h c", h=HL)[:, :, 0:64],
                            in_=ps[:].rearrange("p (h c) -> p h c", h=HL))

                    # attention -> attnT [128, NQ, S] bf16
                    attnT = acts.tile([128, NQ, S], BF16, tag="at")
                    arA = []
                    for c in ([] if skip_attn else range(NCH)):
                        cs = slice(c * CH, (c + 1) * CH)
                        nk = 4 * c + 4
                        QTc = acts.tile([128, NQ, CH], F32R, tag="qt", bufs=2)
                        for m in range(NQ):
                            wl = wtile(wq_d, l, m, NT, F32R)
                            ps = pp.tile([128, CH], F32, tag="pp")
                            for k in range(NT):
                                nc.tensor.matmul(
                                    ps[:], wl[:, k, :], hT[:, k, cs],
                                    start=(k == 0), stop=(k == NT - 1))
                            nc.vector.tensor_scalar_add(
                                out=QTc[:, m, :], in0=ps[:],
                                scalar1=bqT[:, l, m : m + 1])
                        for h in range(HL):
                            base = 64 * (h % 2)
                            hp = h // 2
                            wts = []
                            for j in range(nk):
                                pl = pp.tile([128, CH], F32, tag="pp")
                                nc.tensor.matmul(
                                    pl[:],
                                    KT[base:base + 64, hp, j * 128:(j + 1) * 128],
                                    QTc[base:base + 64, hp, :],
                                    start=True, stop=True)
                                wt = wtp.tile([128, CH], BF16, tag="wt", bufs=8)
                                nc.scalar.activation(out=wt[:], in_=pl[:],
                                                     func=AF.Exp, scale=0.125)
                                r = j - 4 * c
                                if r >= 0:
                                    nc.vector.tensor_tensor(
                                        out=wt[:], in0=wt[:],
                                        in1=masks[:, r, :], op=MULT)
                                wts.append(wt)
                            pa = pav.tile([65, CH], F32, tag="pav")
                            for j in range(nk):
                                nc.tensor.matmul(
                                    pa[:], Vt[:, j, 65 * h : 65 * h + 65],
                                    wts[j][:],
                                    start=(j == 0), stop=(j == nk - 1))
                            rec = rows.tile([1, CH], F32R, tag="rb")
                            nc.vector.reciprocal(out=rec[:], in_=pa[64:65, :])
                            prb2 = pp.tile([128, CH], F32, tag="pp")
                            nc.tensor.matmul(prb2[0:64, :], ones_bc[:, 0:64],
                                             rec[:], start=True, stop=True)
                            avs = wtp.tile([64, CH], BF16, tag="avs", bufs=3)
                            nc.scalar.copy(out=avs[:], in_=pa[0:64, :])
                            nc.vector.tensor_tensor(
                                out=attnT[base:base + 64, hp, cs],
                                in0=avs[:], in1=prb2[0:64, :], op=MULT)
                        for t in range(NQ):
                            nc.vector.tensor_scalar_add(
                                out=attnT[:, t, cs], in0=attnT[:, t, cs],
                                scalar1=bvT[:, l, t : t + 1])
                        # wo partial for this chunk, AR issued immediately;
                        # readback deferred so the collective overlaps the
                        # next chunk's attention / MLP compute.
                        ar_out = dram.tile([128, NT, CH], BF16, tag="arout",
                                           name=f"aroA{c}")
                        ar_in = dram.tile([128, NT, CH], BF16, tag="arin",
                                          name=f"ariA{c}")
                        for m in range(NT):
                            wl = wtile(wo_d, l, m, NQ, BF16)
                            ps = pp.tile([128, CH], F32, tag="pp")
                            for k in range(NQ):
                                nc.tensor.matmul(
                                    ps[:], wl[:, k, :], attnT[:, k, cs],
                                    start=(k == 0), stop=(k == NQ - 1))
                            po = evac.tile([128, CH], BF16, tag="po", bufs=2)
                            nc.vector.tensor_scalar_add(
                                out=po[:], in0=ps[:],
                                scalar1=boT[:, l, m : m + 1])
                            nc.sync.dma_start(out=ar_in[:, m, :], in_=po[:])
                        if not skip_ar:
                            nc.gpsimd.collective_compute(
                                "AllReduce", ADD, replica_groups=REPLICA_GROUPS,
                                ins=[ar_in.opt()], outs=[ar_out.opt()])
                        else:
                            nc.sync.dma_start(out=ar_out[:], in_=ar_in[:])
                        arA.append(ar_out)

                    # residual(x) + LN2 per chunk as each AR_a lands
                    h2c = {}

                    def wr_h2(t, cs, tmp, gs, bs, h2c=h2c):
                        c = cs.start // CH
                        if c not in h2c:
                            h2c[c] = acts.tile([128, NT, CH], BF16,
                                               tag="h2", bufs=2, name=f"h2c{c}")
                        nc.vector.tensor_scalar(
                            out=h2c[c][:, t, :], in0=tmp[:],
                            scalar1=gs, scalar2=bs, op0=MULT, op1=ADD)

                    for c in ([] if skip_attn else range(NCH)):
                        cs = slice(c * CH, (c + 1) * CH)
                        for t in range(NT):
                            ars = evac.tile([128, CH], BF16, tag="ars", bufs=2)
                            nc.sync.dma_start(out=ars[:],
                                              in_=arA[c][:, t, :])
                            nc.vector.tensor_tensor(
                                out=xT[:, t, cs], in0=xT[:, t, cs],
                                in1=ars[:], op=ADD)
                        layernorm_chunk(g2T, be2T, l, wr_h2, c)

                    # MLP per chunk, AR_b issued immediately, readback deferred
                    arB = []
                    for c in ([] if skip_mlp else range(NCH)):
                        cs = slice(c * CH, (c + 1) * CH)
                        gTc = hg.tile([128, NM, CH], BF16, tag="hg",
                                      name=f"gTc{c}")
                        for m in range(NM):
                            wl = wtile(w1_d, l, m, NT, BF16)
                            ps = pp.tile([128, CH], F32, tag="pp")
                            for k in range(NT):
                                nc.tensor.matmul(
                                    ps[:], wl[:, k, :], h2c[c][:, k, :],
                                    start=(k == 0), stop=(k == NT - 1))
                            nc.scalar.activation(
                                out=gTc[:, m, :], in_=ps[:], func=AF.Gelu,
                                bias=b1T[:, l, m : m + 1], scale=1.0)
                        ar_out = dram.tile([128, NT, CH], BF16, tag="arout",
                                           name=f"aroB{c}")
                        ar_in = dram.tile([128, NT, CH], BF16, tag="arin",
                                          name=f"ariB{c}")
                        for m in range(NT):
                            wl = wtile(w2_d, l, m, NM, BF16)
                            ps = pp.tile([128, CH], F32, tag="pp")
                            for k in range(NM):
                                nc.tensor.matmul(
                                    ps[:], wl[:, k, :], gTc[:, k, :],
                                    start=(k == 0), stop=(k == NM - 1))
                            po = evac.tile([128, CH], BF16, tag="po", bufs=2)
                            nc.vector.tensor_scalar_add(
                                out=po[:], in0=ps[:],
                                scalar1=b2T[:, l, m : m + 1])
                            nc.sync.dma_start(out=ar_in[:, m, :], in_=po[:])
                        if not skip_ar:
                            nc.gpsimd.collective_compute(
                                "AllReduce", ADD, replica_groups=REPLICA_GROUPS,
                                ins=[ar_in.opt()], outs=[ar_out.opt()])
                        else:
                            nc.sync.dma_start(out=ar_out[:], in_=ar_in[:])
                        arB.append(ar_out)
                    for c in ([] if skip_mlp else range(NCH)):
                        cs = slice(c * CH, (c + 1) * CH)
                        for t in range(NT):
                            ars = evac.tile([128, CH], BF16, tag="ars", bufs=2)
                            nc.sync.dma_start(out=ars[:],
                                              in_=arB[c][:, t, :])
                            nc.vector.tensor_tensor(
                                out=xT[:, t, cs], in0=xT[:, t, cs],
                                in1=ars[:], op=ADD)

            # final LN -> output (streamed per tile)
            def wr_out(t, cs, tmp, gs, bs):
                ot = evac.tile([128, CH], F32, tag="ot", bufs=2)
                nc.vector.tensor_scalar(out=ot[:], in0=tmp[:],
                                        scalar1=gs, scalar2=bs,
                                        op0=MULT, op1=ADD)
                nc.sync.dma_start(out=out_v[:, t, cs], in_=ot[:])

            layernorm(gfT, befT, None, wr_out)

    return nc


# ---------------------------------------------------------------------------
# host side
# ---------------------------------------------------------------------------

def _sinusoidal_pe(s, d):
    pos = np.arange(s, dtype=np.float32)[:, None]
    div = np.exp(np.arange(0, d, 2, dtype=np.float32)
                 * np.float32(-np.log(10000.0) / d)).astype(np.float32)
    pe = np.zeros((s, d), dtype=np.float32)
    pe[:, 0::2] = np.sin(pos * div)
    pe[:, 1::2] = np.cos(pos * div)
    return pe


def _pp128(v):
    """[L?, n*128] -> [L?, 128, n] with feature = 128*m + p."""
    v = np.asarray(v, dtype=np.float32)
    if v.ndim == 1:
        return np.ascontiguousarray(v.reshape(-1, 128).T)
    lq, n = v.shape
    return np.ascontiguousarray(v.reshape(lq, n // 128, 128).transpose(0, 2, 1))


_NC_CACHE = {}


def _get_nc(repeat=1):
    if repeat not in _NC_CACHE:
        _NC_CACHE[repeat] = build_nc(repeat)
    return _NC_CACHE[repeat]


def make_in_maps(input_ids, tok_emb, wq, bq, wk, bk, wv, bv, wo, bo,
                 ln1_g, ln1_b, ln2_g, ln2_b, w1, b1, w2, b2, lnf_g, lnf_b):
    input_ids = np.asarray(input_ids)
    pe = _sinusoidal_pe(S, D)
    masks = np.zeros((128, 4, CH), dtype=np.float32)
    ar = np.arange(CH)
    for r in range(4):
        for p in range(128):
            masks[p, r, :] = (ar >= 128 * r + p).astype(np.float32)
    cones = np.ones((128, 128), dtype=np.float32)

    in_maps = []
    for core in range(N_CORES):
        b = core // 2
        j = core % 2
        qs = slice(j * SH, (j + 1) * SH)
        fs = slice(j * FSH, (j + 1) * FSH)
        x0 = (tok_emb[input_ids[b]] + pe).astype(np.float32)   # [S, D]
        m = {
            "x0t": np.ascontiguousarray(x0.T),
            "wq": np.ascontiguousarray(wq[:, :, qs]),
            "wk": np.ascontiguousarray(wk[:, :, qs]),
            "wv": np.ascontiguousarray(wv[:, :, qs]),
            "wo": np.ascontiguousarray(wo[:, qs, :]).astype(ml_dtypes.bfloat16),
            "w1": np.ascontiguousarray(w1[:, :, fs]).astype(ml_dtypes.bfloat16),
            "w2": np.ascontiguousarray(w2[:, fs, :]).astype(ml_dtypes.bfloat16),
            "bq": _pp128(bq[:, qs]),
            "bk": _pp128(bk[:, qs]),
            "bv": _pp128(bv[:, qs]),
            "bo2": _pp128(bo * 0.5),
            "b1": _pp128(b1[:, fs]),
            "b22": _pp128(b2 * 0.5),
            "g1": _pp128(ln1_g),
            "be1": _pp128(ln1_b),
            "g2": _pp128(ln2_g),
            "be2": _pp128(ln2_b),
            "gf": _pp128(lnf_g),
            "bef": _pp128(lnf_b),
            "masks": masks,
            "cones": cones,
        }
        in_maps.append(m)
    return in_maps


def kernel(input_ids, attention_mask, tok_emb, ln1_g, ln1_b, wq, bq, wk, bk,
           wv, bv, wo, bo, ln2_g, ln2_b, w1, b1, w2, b2, lnf_g, lnf_b,
           _repeat=1):
    args = [np.asarray(a, dtype=np.float32) for a in
            (tok_emb, wq, bq, wk, bk, wv, bv, wo, bo,
             ln1_g, ln1_b, ln2_g, ln2_b, w1, b1, w2, b2, lnf_g, lnf_b)]
    (tok_emb, wq, bq, wk, bk, wv, bv, wo, bo,
     ln1_g, ln1_b, ln2_g, ln2_b, w1, b1, w2, b2, lnf_g, lnf_b) = args
    in_maps = make_in_maps(input_ids, tok_emb, wq, bq, wk, bk, wv, bv, wo, bo,
                           ln1_g, ln1_b, ln2_g, ln2_b, w1, b1, w2, b2,
                           lnf_g, lnf_b)
    nc = _get_nc(_repeat)
    res = run_bass_kernel_spmd(nc, in_maps, list(range(N_CORES)))
    out = np.empty((B, S, D), dtype=np.float32)
    for b in range(B):
        out[b] = res.results[2 * b]["outt"].T
    return out
